# revision 1
# baseline (speedup 1.0000x reference)
"""LSTM-cell (shared-gate) Trainium2 kernel.

Reference computes, for B=8192, IN=H=4096:
    z = x @ Wi.T + bi + h @ Wh.T + bh        # [B, H]
    s = sigmoid(z); g = tanh(z)
    c_new = c*s + s*g = s*(c+g)
    out = s*tanh(c_new)
    returns (out, c_new)

Strategy: data-parallel over batch across 8 NeuronCores (B_local=1024).
On each core one fused matmul z.T = [Wi;Wh].T^T @ [x;h].T with K=8192,
computed in transposed orientation (partition dim = hidden) so the
per-partition gate biases ride the ScalarE activation's bias operand.
Matmuls run in bf16 (full PE rate, 1 cyc/row); accumulation + all gate
math in fp32.  Host pre-transposes/casts/retiles inputs (untimed).
"""

import os
import sys
import time

import numpy as np

if "/opt/trn_rl_repo" not in sys.path:
    sys.path.insert(0, "/opt/trn_rl_repo")

import ml_dtypes

import concourse.bass as bass
import concourse.mybir as mybir
from concourse import bacc
from concourse.tile import TileContext
from concourse.bass_utils import run_bass_kernel_spmd

B, IN, H = 8192, 4096, 4096
NCORES = 8
BL = B // NCORES          # 1024 batch rows per core
K = IN + H                # 8192 contraction
KS = K // 128             # 64 k-stripes
MBLK = H // 128           # 32 output-partition blocks
NB = BL // 512            # 2 free-dim chunks of 512

BF16 = mybir.dt.bfloat16
F32 = mybir.dt.float32
AF = mybir.ActivationFunctionType

_cache = {}


def _build_nc(reps=1):
    nc = bacc.Bacc("TRN2", target_bir_lowering=False)

    xh = nc.dram_tensor("xh", [K, BL], BF16, kind="ExternalInput")
    w = nc.dram_tensor("w", [MBLK, 128, KS * 128], BF16, kind="ExternalInput")
    bias = nc.dram_tensor("bias", [128, MBLK], F32, kind="ExternalInput")
    ct = nc.dram_tensor("ct", [H, BL], F32, kind="ExternalInput")
    outT = nc.dram_tensor("outT", [H, BL], F32, kind="ExternalOutput")
    cnewT = nc.dram_tensor("cnewT", [H, BL], F32, kind="ExternalOutput")

    with TileContext(nc) as tc:
        with (
            tc.tile_pool(name="xpool", bufs=1) as xpool,
            tc.tile_pool(name="wpool", bufs=2) as wpool,
            tc.tile_pool(name="bpool", bufs=1) as bpool,
            tc.tile_pool(name="cpool", bufs=2) as cpool,
            tc.tile_pool(name="spool", bufs=2) as spool,
            tc.tile_pool(name="gpool", bufs=2) as gpool,
            tc.tile_pool(name="cnpool", bufs=2) as cnpool,
            tc.tile_pool(name="upool", bufs=2) as upool,
            tc.tile_pool(name="opool", bufs=2) as opool,
            tc.tile_pool(name="psum", bufs=4, space="PSUM") as psum_pool,
        ):
            bias_sb = bpool.tile([128, MBLK], F32)
            nc.sync.dma_start(out=bias_sb[:], in_=bias[:])

            # Entire [x;h].T slab stays resident: [128, 64, 1024] bf16 = 128KB/part
            X_sb = xpool.tile([128, KS, BL], BF16)
            xh_r = xh.rearrange("(ks p) b -> p ks b", p=128)
            CH = 8
            for c0 in range(0, KS, CH):
                nc.sync.dma_start(
                    out=X_sb[:, c0 : c0 + CH, :], in_=xh_r[:, c0 : c0 + CH, :]
                )

            for m in [mm for _ in range(reps) for mm in range(MBLK)]:
                w_sb = wpool.tile([128, KS * 128], BF16)
                nc.sync.dma_start(out=w_sb[:], in_=w[m])

                ps = [psum_pool.tile([128, 512], F32, name=f"ps{n}") for n in range(NB)]
                for k in range(KS):
                    lhsT = w_sb[:, k * 128 : (k + 1) * 128]
                    for n in range(NB):
                        nc.tensor.matmul(
                            ps[n][:],
                            lhsT,
                            X_sb[:, k, n * 512 : (n + 1) * 512],
                            start=(k == 0),
                            stop=(k == KS - 1),
                        )

                bvec = bias_sb[:, m : m + 1]
                for n in range(NB):
                    rs = slice(m * 128, (m + 1) * 128)
                    cs = slice(n * 512, (n + 1) * 512)
                    c_t = cpool.tile([128, 512], F32)
                    nc.sync.dma_start(out=c_t[:], in_=ct[rs, cs])

                    s_t = spool.tile([128, 512], F32)
                    g_t = gpool.tile([128, 512], F32)
                    nc.scalar.activation(s_t[:], ps[n][:], AF.Sigmoid, bias=bvec)
                    nc.scalar.activation(g_t[:], ps[n][:], AF.Tanh, bias=bvec)

                    nc.vector.tensor_add(g_t[:], g_t[:], c_t[:])  # g = c + g
                    cn_t = cnpool.tile([128, 512], F32)
                    nc.vector.tensor_mul(cn_t[:], g_t[:], s_t[:])  # c_new = s*(c+g)
                    u_t = upool.tile([128, 512], F32)
                    nc.scalar.activation(u_t[:], cn_t[:], AF.Tanh)
                    o_t = opool.tile([128, 512], F32)
                    nc.vector.tensor_mul(o_t[:], u_t[:], s_t[:])  # out = s*tanh(c_new)

                    nc.sync.dma_start(out=cnewT[rs, cs], in_=cn_t[:])
                    nc.sync.dma_start(out=outT[rs, cs], in_=o_t[:])

    nc.finalize()
    return nc


def _prep_inputs(x, h, c, Wi, bi, Wh, bh):
    bf = ml_dtypes.bfloat16
    x = np.asarray(x, np.float32)
    h = np.asarray(h, np.float32)
    c = np.asarray(c, np.float32)
    Wi = np.asarray(Wi, np.float32)
    Wh = np.asarray(Wh, np.float32)

    xhT = np.empty((K, B), dtype=bf)
    xhT[:IN] = x.T
    xhT[IN:] = h.T

    WT = np.empty((K, H), dtype=np.float32)
    WT[:IN] = Wi.T
    WT[IN:] = Wh.T
    # Wre[m, p, k*128+j] = WT[k*128+p, m*128+j] -> each [128, 8192] block is
    # one m-slice with 16KB contiguous per partition.
    Wre = np.ascontiguousarray(
        WT.reshape(KS, 128, MBLK, 128).transpose(2, 1, 0, 3).reshape(MBLK, 128, KS * 128)
    ).astype(bf)

    bias_re = np.ascontiguousarray(
        (np.asarray(bi, np.float32) + np.asarray(bh, np.float32))
        .reshape(MBLK, 128)
        .T
    )

    cT = np.ascontiguousarray(c.T)  # [H, B]

    in_maps = []
    for cid in range(NCORES):
        bs = slice(cid * BL, (cid + 1) * BL)
        in_maps.append(
            {
                "xh": np.ascontiguousarray(xhT[:, bs]),
                "w": Wre,
                "bias": bias_re,
                "ct": np.ascontiguousarray(cT[:, bs]),
            }
        )
    return in_maps


def kernel(x, h, c, Wi, bi, Wh, bh):
    if "nc" not in _cache:
        _cache["nc"] = _build_nc()
    nc = _cache["nc"]

    in_maps = _prep_inputs(x, h, c, Wi, bi, Wh, bh)
    res = run_bass_kernel_spmd(nc, in_maps, core_ids=list(range(NCORES)))

    outT = np.concatenate([r["outT"] for r in res.results], axis=1)  # [H, B]
    cnewT = np.concatenate([r["cnewT"] for r in res.results], axis=1)
    out = np.ascontiguousarray(outT.T, dtype=np.float32)
    c_new = np.ascontiguousarray(cnewT.T, dtype=np.float32)
    return (out, c_new)


def benchmark(x, h, c, Wi, bi, Wh, bh, iters=3):
    """Device-resident timing: stage inputs once, run `iters` donated execs."""
    import jax
    from jax.sharding import Mesh, PartitionSpec, NamedSharding
    from jax.experimental.shard_map import shard_map
    from concourse import bass2jax

    if "nc" not in _cache:
        _cache["nc"] = _build_nc()
    nc = _cache["nc"]
    in_maps = _prep_inputs(x, h, c, Wi, bi, Wh, bh)

    bass2jax.install_neuronx_cc_hook()

    partition_name = nc.partition_id_tensor.name if nc.partition_id_tensor else None
    in_names, out_names, out_avals, zero_shapes = [], [], [], []
    for alloc in nc.m.functions[0].allocations:
        if not isinstance(alloc, mybir.MemoryLocationSet):
            continue
        name = alloc.memorylocations[0].name
        if alloc.kind == "ExternalInput":
            if name != partition_name:
                in_names.append(name)
        elif alloc.kind == "ExternalOutput":
            out_names.append(name)
            shape = tuple(alloc.tensor_shape)
            dtype = mybir.dt.np(alloc.dtype)
            out_avals.append(jax.core.ShapedArray(shape, dtype))
            zero_shapes.append((shape, dtype))
    n_params = len(in_names)
    n_outs = len(out_names)
    all_in_names = list(in_names) + list(out_names)
    if partition_name is not None:
        all_in_names.append(partition_name)

    donate = tuple(range(n_params, n_params + n_outs))

    def _body(*args):
        operands = list(args)
        if partition_name is not None:
            operands.append(bass2jax.partition_id_tensor())
        outs = bass2jax._bass_exec_p.bind(
            *operands,
            out_avals=tuple(out_avals),
            in_names=tuple(all_in_names),
            out_names=tuple(out_names),
            lowering_input_output_aliases=(),
            sim_require_finite=True,
            sim_require_nnan=True,
            nc=nc,
        )
        return tuple(outs)

    devices = jax.devices()[:NCORES]
    mesh = Mesh(np.asarray(devices), ("core",))
    sh = NamedSharding(mesh, PartitionSpec("core"))
    n_all = n_params + n_outs
    sharded = jax.jit(
        shard_map(
            _body,
            mesh=mesh,
            in_specs=(PartitionSpec("core"),) * n_all,
            out_specs=(PartitionSpec("core"),) * n_outs,
            check_rep=False,
        ),
        donate_argnums=donate,
        keep_unused=True,
    )

    concat_in = [
        np.concatenate([np.asarray(in_maps[cid][nm]) for cid in range(NCORES)], axis=0)
        for nm in in_names
    ]
    dev_in = [jax.device_put(a, sh) for a in concat_in]

    def zero_set():
        return [
            jax.device_put(np.zeros((NCORES * s[0], *s[1:]), d), sh)
            for (s, d) in zero_shapes
        ]

    zsets = [zero_set() for _ in range(iters + 1)]
    outs = sharded(*dev_in, *zsets[0])
    jax.block_until_ready(outs)
    t0 = time.perf_counter()
    for i in range(iters):
        outs = sharded(*dev_in, *zsets[1 + i])
        jax.block_until_ready(outs)
    t1 = time.perf_counter()
    dur_ns = (t1 - t0) / iters * 1e9

    outT = np.asarray(outs[out_names.index("outT")]).reshape(NCORES, H, BL)
    cnewT = np.asarray(outs[out_names.index("cnewT")]).reshape(NCORES, H, BL)
    outT = np.concatenate(list(outT), axis=1)
    cnewT = np.concatenate(list(cnewT), axis=1)
    out = np.ascontiguousarray(outT.T, dtype=np.float32)
    c_new = np.ascontiguousarray(cnewT.T, dtype=np.float32)
    return dur_ns, (out, c_new)



# revision 2
# speedup vs baseline: 81.1875x; 81.1875x over previous
"""LSTM-cell (shared-gate) Trainium2 kernel.

Reference computes, for B=8192, IN=H=4096:
    z = x @ Wi.T + bi + h @ Wh.T + bh        # [B, H]
    s = sigmoid(z); g = tanh(z)
    c_new = c*s + s*g = s*(c+g)
    out = s*tanh(c_new)
    returns (out, c_new)

Strategy: data-parallel over batch across 8 NeuronCores (B_local=1024).
Each core runs one fused matmul z.T = [Wi;Wh].T^T @ [x;h].T with K=8192 in
transposed orientation (partition dim = hidden) so the per-partition gate
biases ride the ScalarE activation's bias operand.  Matmuls in bf16
(full PE rate), accumulation + gate math in fp32; c/out are bf16 at the
DRAM boundary (error budget is ample).

Perf-critical structure (HW-measured on trn2):
- X resident in SBUF ([128, 64, 1024] bf16), loaded via 8 chunked DMAs on
  the SP HWDGE ring so the first matmuls start ~12us in.
- Weights streamed on the *Activation* HWDGE ring (own FIFO ring, half-block
  chunks, 6-deep pool) -- sharing a ring with the X stream or epilogue
  serializes the PE behind DMA waits (costs ~45%).
- Epilogue DMAs (c in, out/c_new out) on SWDGE (gpsimd) -- a third,
  independent DMA path.
- PSUM: 4 generations x 2 banks = all 8 banks, so the tensor engine streams
  accumulation groups back-to-back while ScalarE/DVE drain older groups.
"""

import sys

if "/opt/trn_rl_repo" not in sys.path:
    sys.path.insert(0, "/opt/trn_rl_repo")

import numpy as np
import ml_dtypes

import concourse.bass as bass
import concourse.mybir as mybir
from concourse import bacc
from concourse.tile import TileContext
from concourse.bass_utils import run_bass_kernel_spmd

B, IN, H = 8192, 4096, 4096
NCORES = 8
BL = B // NCORES          # 1024 batch rows per core
K = IN + H                # 8192 contraction
KS = K // 128             # 64 k-stripes
MBLK = H // 128           # 32 output-partition blocks
NB = BL // 512            # 2 psum tiles of 512 per m-block

BF16 = mybir.dt.bfloat16
F32 = mybir.dt.float32
AF = mybir.ActivationFunctionType

_cache = {}


def _build_nc(reps=1, wbufs=6, wsplit=2, xchunks=8):
    nc = bacc.Bacc("TRN2", target_bir_lowering=False)

    xh = nc.dram_tensor("xh2", [128, KS * BL], BF16, kind="ExternalInput")
    w = nc.dram_tensor("w", [MBLK, 128, KS * 128], BF16, kind="ExternalInput")
    bias = nc.dram_tensor("bias", [128, MBLK], F32, kind="ExternalInput")
    ct = nc.dram_tensor("ct2", [MBLK, 128, BL], BF16, kind="ExternalInput")
    outT = nc.dram_tensor("outP", [MBLK, 128, BL], BF16, kind="ExternalOutput")
    cnewT = nc.dram_tensor("cnewP", [MBLK, 128, BL], BF16, kind="ExternalOutput")

    KSC = KS // wsplit

    with TileContext(nc) as tc:
        with (
            tc.tile_pool(name="xpool", bufs=1) as xpool,
            tc.tile_pool(name="wpool", bufs=wbufs) as wpool,
            tc.tile_pool(name="bpool", bufs=1) as bpool,
            tc.tile_pool(name="cpool", bufs=2) as cpool,
            tc.tile_pool(name="spool", bufs=2) as spool,
            tc.tile_pool(name="gpool", bufs=2) as gpool,
            tc.tile_pool(name="cnpool", bufs=2) as cnpool,
            tc.tile_pool(name="upool", bufs=2) as upool,
            tc.tile_pool(name="opool", bufs=2) as opool,
            tc.tile_pool(name="psum", bufs=4, space="PSUM") as psum_pool,
        ):
            bias_sb = bpool.tile([128, MBLK], F32)
            nc.sync.dma_start(out=bias_sb[:], in_=bias[:])

            for rep in range(reps):
                X_sb = xpool.tile([128, KS, BL], BF16)
                xv = xh[:].rearrange("p (ks b) -> p ks b", ks=KS)
                KC = KS // xchunks
                for c0 in range(0, KS, KC):
                    nc.sync.dma_start(out=X_sb[:, c0:c0 + KC, :], in_=xv[:, c0:c0 + KC, :])

                for m in range(MBLK):
                    chunks = []
                    for s in range(wsplit):
                        w_sb = wpool.tile([128, KSC * 128], BF16)
                        nc.scalar.dma_start(
                            out=w_sb[:],
                            in_=w[m, :, s * KSC * 128:(s + 1) * KSC * 128])
                        chunks.append(w_sb)

                    ps = [psum_pool.tile([128, 512], F32, name=f"ps{n}") for n in range(NB)]
                    for k in range(KS):
                        lhsT = chunks[k // KSC][:, (k % KSC) * 128:(k % KSC + 1) * 128]
                        for n in range(NB):
                            nc.tensor.matmul(
                                ps[n][:], lhsT,
                                X_sb[:, k, n * 512:(n + 1) * 512],
                                start=(k == 0), stop=(k == KS - 1))

                    c_t = cpool.tile([128, BL], BF16)
                    nc.gpsimd.dma_start(out=c_t[:], in_=ct[m])
                    cn_t = cnpool.tile([128, BL], BF16)
                    o_t = opool.tile([128, BL], BF16)
                    bvec = bias_sb[:, m:m + 1]
                    for n in range(NB):
                        cs = slice(n * 512, (n + 1) * 512)
                        s_t = spool.tile([128, 512], F32)
                        g_t = gpool.tile([128, 512], F32)
                        nc.scalar.activation(s_t[:], ps[n][:], AF.Sigmoid, bias=bvec)
                        nc.scalar.activation(g_t[:], ps[n][:], AF.Tanh, bias=bvec)
                        nc.vector.tensor_add(g_t[:], g_t[:], c_t[:, cs])   # g = c + g
                        nc.vector.tensor_mul(cn_t[:, cs], g_t[:], s_t[:])  # c_new = s*(c+g)
                        u_t = upool.tile([128, 512], F32)
                        nc.scalar.activation(u_t[:], cn_t[:, cs], AF.Tanh)
                        nc.vector.tensor_mul(o_t[:, cs], u_t[:], s_t[:])   # out = s*tanh
                    nc.gpsimd.dma_start(out=cnewT[m], in_=cn_t[:])
                    nc.gpsimd.dma_start(out=outT[m], in_=o_t[:])

    nc.finalize()
    return nc


def _prep_inputs(x, h, c, Wi, bi, Wh, bh):
    bf = ml_dtypes.bfloat16
    x = np.asarray(x, np.float32)
    h = np.asarray(h, np.float32)
    c = np.asarray(c, np.float32)

    # [x;h].T in bf16, retiled so each partition's stripe data is contiguous
    xhT = np.empty((K, B), dtype=bf)
    xhT[:IN] = x.T
    xhT[IN:] = h.T

    WT = np.empty((K, H), dtype=np.float32)
    WT[:IN] = np.asarray(Wi, np.float32).T
    WT[IN:] = np.asarray(Wh, np.float32).T
    # Wre[m, p, k*128+j] = WT[k*128+p, m*128+j]: each [128, 8192] m-slice has
    # 16KB contiguous per partition.
    Wre = np.ascontiguousarray(
        WT.reshape(KS, 128, MBLK, 128).transpose(2, 1, 0, 3).reshape(MBLK, 128, KS * 128)
    ).astype(bf)

    bias_re = np.ascontiguousarray(
        (np.asarray(bi, np.float32) + np.asarray(bh, np.float32)).reshape(MBLK, 128).T
    )

    cT = c.T  # [H, B]
    in_maps = []
    for cid in range(NCORES):
        bs = slice(cid * BL, (cid + 1) * BL)
        xh_s = np.ascontiguousarray(xhT[:, bs])
        xh2 = np.ascontiguousarray(
            xh_s.reshape(KS, 128, BL).transpose(1, 0, 2).reshape(128, KS * BL))
        ct2 = np.ascontiguousarray(cT[:, bs].reshape(MBLK, 128, BL)).astype(bf)
        in_maps.append({"xh2": xh2, "w": Wre, "bias": bias_re, "ct2": ct2})
    return in_maps


def _unpack(res_list):
    outs, cns = [], []
    for r in res_list:
        outs.append(np.asarray(r["outP"], np.float32).reshape(H, BL))
        cns.append(np.asarray(r["cnewP"], np.float32).reshape(H, BL))
    outT = np.concatenate(outs, axis=1)   # [H, B]
    cnewT = np.concatenate(cns, axis=1)
    return (np.ascontiguousarray(outT.T), np.ascontiguousarray(cnewT.T))


def kernel(x, h, c, Wi, bi, Wh, bh):
    if "nc" not in _cache:
        _cache["nc"] = _build_nc()
    nc = _cache["nc"]

    in_maps = _prep_inputs(x, h, c, Wi, bi, Wh, bh)
    res = run_bass_kernel_spmd(nc, in_maps, core_ids=list(range(NCORES)))
    return _unpack(res.results)
